# revision 1
# baseline (speedup 1.0000x reference)
"""Trainium2 Bass kernel for GraphTripletGCNLayer.

Reference computation (N=100000 nodes, R=100000 rels, T=300000 triples, H=256):
    rel = rel_states[rel_idx]
    agg = zeros; agg[obj] += node[subj] + rel; agg[subj] += node[obj] + rel
    out = node + silu(concat([node, agg]) @ W + b)

Strategy (8 cores, dst-node sharded):
  - Each core owns a contiguous slab of ~N/8 destination nodes.
  - node/rel tables are replicated (bf16) in each core's DRAM; per-message
    source rows are fetched with gpsimd.dma_gather (int16 indices -> tables
    are addressed in <=32768-row chunks; messages are host-sorted by
    (group-of-windows, chunk, window) per stream).
  - Aggregation avoids scatter entirely: for each 128-dst-row window,
    agg^T accumulates in PSUM as sum over 128-message tiles of
    msg_tile^T @ onehot(d) matmuls (onehot built on DVE via iota==d).
    This yields agg already feature-major, so no transposes are needed.
  - Projection: y^T = W^T x^T over [node^T; agg^T] in bf16, then
    silu(+bias) on ScalarE, f32 residual add, and a feature-major f32
    output which the host transposes back.
"""

import sys

sys.path.insert(0, "/opt/trn_rl_repo")

import numpy as np
import ml_dtypes

import concourse.bass as bass
import concourse.bacc as bacc
import concourse.mybir as mybir
import concourse.tile as tile
from concourse.bass_utils import run_bass_kernel_spmd

BF16 = mybir.dt.bfloat16
F32 = mybir.dt.float32
I16 = mybir.dt.int16

NCORES = 8
WIN = 128          # dst rows per window (= PSUM partition count of onehot mm)
GW = 4             # windows per group (projection granularity: 512 nodes)


def _ceil(a, b):
    return -(-a // b)


def _plan(node_states, rel_states, triples):
    """Host-side message planning. Returns cfg dict + per-core arrays."""
    N, H = node_states.shape
    R = rel_states.shape[0]
    T = triples.shape[0]
    assert H == 256, H

    OWN = _ceil(N, NCORES)           # owned dst nodes per core
    WPC = _ceil(OWN, WIN)            # real windows per core
    NG = _ceil(WPC, GW)              # groups per core
    NPAD = NG * GW * WIN             # padded node columns per core

    tr = np.asarray(triples).astype(np.int64)
    s, r, o = tr[:, 0], tr[:, 1], tr[:, 2]
    # messages: (src_node, rel, dst)
    src = np.concatenate([s, o])
    rel = np.concatenate([r, r])
    dst = np.concatenate([o, s])
    owner = dst // OWN
    dl = dst - owner * OWN
    w = dl // WIN
    d = (dl - w * WIN).astype(np.float32)

    streams = {}
    for name, gidx, tabrows in (("n", src, N), ("r", rel, R)):
        NCH = _ceil(tabrows, 32768)
        CH = _ceil(tabrows, NCH)
        chunk = gidx // CH
        lidx = (gidx - chunk * CH).astype(np.int16)
        # counts[owner, chunk, window]
        counts = np.zeros((NCORES, NCH, WPC), dtype=np.int64)
        np.add.at(counts, (owner, chunk, w), 1)
        K = _ceil(counts.max(axis=0), WIN).astype(np.int64)  # [NCH, WPC] tiles
        # layout order: for g: for c: for w in group  -> rank R[c,w]
        rank = np.zeros((NCH, WPC), dtype=np.int64)
        order = []
        for g in range(NG):
            for c in range(NCH):
                for wi in range(g * GW, min((g + 1) * GW, WPC)):
                    rank[c, wi] = len(order)
                    order.append((c, wi))
        ntile_by_rank = np.array([K[c, wi] for (c, wi) in order], dtype=np.int64)
        tile_base_by_rank = np.concatenate([[0], np.cumsum(ntile_by_rank)[:-1]])
        slot_base_by_rank = tile_base_by_rank * WIN
        T_tiles = int(ntile_by_rank.sum())
        S = T_tiles * WIN

        # per-core slot assignment
        idx_cores = np.zeros((NCORES, 128, S // 16), dtype=np.int16)
        dcol_cores = np.full((NCORES, 128, max(T_tiles, 1)), -1.0,
                             dtype=np.float32)
        mrank = rank[chunk, w]
        for core in range(NCORES):
            m = owner == core
            mr = mrank[m]
            ml_ = lidx[m]
            md = d[m]
            srt = np.argsort(mr, kind="stable")
            mr = mr[srt]
            ml_ = ml_[srt]
            md = md[srt]
            # rank-run starts
            starts = np.searchsorted(mr, np.arange(len(order)))
            pos_in_run = np.arange(mr.size) - starts[mr]
            slots = slot_base_by_rank[mr] + pos_in_run
            idx_flat = np.zeros(S, dtype=np.int16)
            d_flat = np.full(S, -1.0, dtype=np.float32)
            idx_flat[slots] = ml_
            d_flat[slots] = md
            idx_cores[core] = np.tile(idx_flat.reshape(-1, 16).T, (8, 1))
            dcol_cores[core] = (
                d_flat.reshape(T_tiles, WIN).T.astype(np.float32))

        streams[name] = dict(
            NCH=NCH, CH=CH, K=K, rank=rank,
            tile_base_by_rank=tile_base_by_rank, order=order,
            T_tiles=T_tiles, S=S, idx=idx_cores, dcol=dcol_cores,
        )

    cfg = dict(N=N, R=R, H=H, T=T, OWN=OWN, WPC=WPC, NG=NG, NPAD=NPAD,
               streams=streams)
    return cfg


def _build_program(cfg):
    N, R, H = cfg["N"], cfg["R"], cfg["H"]
    WPC, NG, NPAD = cfg["WPC"], cfg["NG"], cfg["NPAD"]
    stn, str_ = cfg["streams"]["n"], cfg["streams"]["r"]

    nc = bacc.Bacc("TRN2", target_bir_lowering=False, debug=False)

    tab_n = nc.dram_tensor("tab_n", [N, H], BF16, kind="ExternalInput")
    tab_r = nc.dram_tensor("tab_r", [R, H], BF16, kind="ExternalInput")
    idx_n = nc.dram_tensor("idx_n", [128, stn["S"] // 16], I16,
                           kind="ExternalInput")
    idx_r = nc.dram_tensor("idx_r", [128, str_["S"] // 16], I16,
                           kind="ExternalInput")
    dcol_n = nc.dram_tensor("dcol_n", [128, max(stn["T_tiles"], 1)], F32,
                            kind="ExternalInput")
    dcol_r = nc.dram_tensor("dcol_r", [128, max(str_["T_tiles"], 1)], F32,
                            kind="ExternalInput")
    ndT16 = nc.dram_tensor("ndT16", [2, 128, NPAD], BF16, kind="ExternalInput")
    ndT32 = nc.dram_tensor("ndT32", [2, 128, NPAD], F32, kind="ExternalInput")
    w_blk = nc.dram_tensor("w_blk", [128, 8 * 128], BF16, kind="ExternalInput")
    b_blk = nc.dram_tensor("b_blk", [128, 2], F32, kind="ExternalInput")
    iota_d = nc.dram_tensor("iota_d", [128, 128], BF16, kind="ExternalInput")
    yT = nc.dram_tensor("yT", [2, 128, NPAD], F32, kind="ExternalOutput")

    dram = dict(n=(tab_n, idx_n, dcol_n), r=(tab_r, idx_r, dcol_r))
    NWIN_GRP = GW * WIN  # node columns per group

    with tile.TileContext(nc) as tc:
        with (
            tc.tile_pool(name="const", bufs=1) as cpool,
            tc.tile_pool(name="meta", bufs=1) as mpool,
            tc.tile_pool(name="gath", bufs=2) as gpool,
            tc.tile_pool(name="oh", bufs=6) as ohpool,
            tc.tile_pool(name="aggT", bufs=2) as apool,
            tc.tile_pool(name="ndt", bufs=2) as npool,
            tc.tile_pool(name="eout", bufs=2) as epool,
            tc.tile_pool(name="pswin", bufs=3, space="PSUM") as pswin,
            tc.tile_pool(name="psy", bufs=2, space="PSUM") as psy,
        ):
            iota_sb = cpool.tile([128, 128], BF16)
            nc.sync.dma_start(iota_sb[:], iota_d[:])
            w_sb = cpool.tile([128, 8 * 128], BF16)
            nc.sync.dma_start(w_sb[:], w_blk[:])
            b_sb = cpool.tile([128, 2], F32)
            nc.sync.dma_start(b_sb[:], b_blk[:])

            meta = {}
            for sname, st in (("n", stn), ("r", str_)):
                _, idx_t, dcol_t = dram[sname]
                idx_sb = mpool.tile([128, st["S"] // 16], I16, tag=f"idx{sname}",
                                    name=f"idx{sname}")
                nc.sync.dma_start(idx_sb[:], idx_t[:])
                dcol_sb = mpool.tile([128, max(st["T_tiles"], 1)], F32,
                                     tag=f"dc{sname}", name=f"dc{sname}")
                nc.sync.dma_start(dcol_sb[:], dcol_t[:])
                meta[sname] = (idx_sb, dcol_sb)

            for g in range(NG):
                wins = list(range(g * GW, min((g + 1) * GW, WPC)))
                # ---- gathers for this group ----
                gt = {}
                for sname, st in (("n", stn), ("r", str_)):
                    tab_t, _, _ = dram[sname]
                    idx_sb, _ = meta[sname]
                    K = st["K"]
                    Tg = int(K[:, wins].sum())
                    if Tg == 0:
                        gt[sname] = None
                        continue
                    gtile = gpool.tile([128, Tg, H], BF16, tag=f"g{sname}",
                                       name=f"g{sname}")
                    off = 0
                    MAXK = 4  # <=512 idxs/call: SWDGE desc ring limit
                    for c in range(st["NCH"]):
                        Kc = int(K[c, wins].sum())
                        if Kc == 0:
                            continue
                        r0 = st["rank"][c, wins[0]]
                        slot0 = int(st["tile_base_by_rank"][r0]) * WIN
                        rows0 = c * st["CH"]
                        rows1 = min(st["CH"] * (c + 1),
                                    N if sname == "n" else R)
                        done = 0
                        while done < Kc:
                            kk = min(MAXK, Kc - done)
                            nidx = kk * WIN
                            s0 = slot0 + done * WIN
                            nc.gpsimd.dma_gather(
                                gtile[:, off + done:off + done + kk, :],
                                tab_t[rows0:rows1, :],
                                idx_sb[:, s0 // 16:(s0 + nidx) // 16],
                                nidx, nidx, H,
                            )
                            done += kk
                        off += Kc
                    gt[sname] = gtile

                # ---- per-window onehot matmuls -> aggT ----
                aggT = [apool.tile([128, NWIN_GRP], BF16, tag=f"aggT{m}",
                                   name=f"aggT{m}") for m in range(2)]
                for wi, wv in enumerate(wins):
                    # collect tiles for this window: (stream, group-local tile,
                    # global tile index)
                    tl = []
                    for sname, st in (("n", stn), ("r", str_)):
                        K = st["K"]
                        off = 0
                        for c in range(st["NCH"]):
                            for wj in wins:
                                k = int(K[c, wj])
                                if wj == wv:
                                    tb = int(st["tile_base_by_rank"][
                                        st["rank"][c, wj]])
                                    for t in range(k):
                                        tl.append((sname, off + t, tb + t))
                                off += k
                    if not tl:
                        continue
                    pw = [pswin.tile([128, WIN], F32, tag=f"pw{m}",
                                     name=f"pw{m}") for m in range(2)]
                    for i, (sname, tloc, tglob) in enumerate(tl):
                        _, dcol_sb = meta[sname]
                        oh = ohpool.tile([128, WIN], BF16, tag="oh")
                        nc.vector.tensor_scalar(
                            oh[:], iota_sb[:], dcol_sb[:, tglob:tglob + 1],
                            None, mybir.AluOpType.is_equal)
                        for m in range(2):
                            nc.tensor.matmul(
                                pw[m][:],
                                lhsT=gt[sname][:, tloc, m * 128:(m + 1) * 128],
                                rhs=oh[:],
                                start=(i == 0), stop=(i == len(tl) - 1))
                    for m in range(2):
                        nc.vector.tensor_copy(
                            aggT[m][:, wi * WIN:(wi + 1) * WIN], pw[m][:])

                # ---- projection + epilogue ----
                col0 = g * NWIN_GRP
                nt16 = []
                nt32 = []
                for m in range(2):
                    t16 = npool.tile([128, NWIN_GRP], BF16, tag=f"nt16_{m}",
                                     name=f"nt16_{m}")
                    nc.sync.dma_start(t16[:], ndT16[m, :, col0:col0 + NWIN_GRP])
                    nt16.append(t16)
                    t32 = npool.tile([128, NWIN_GRP], F32, tag=f"nt32_{m}",
                                     name=f"nt32_{m}")
                    nc.sync.dma_start(t32[:], ndT32[m, :, col0:col0 + NWIN_GRP])
                    nt32.append(t32)
                for m in range(2):
                    py = psy.tile([128, NWIN_GRP], F32)
                    for k in range(4):
                        rhs = nt16[k] if k < 2 else aggT[k - 2]
                        kb = k * 2 + m
                        nc.tensor.matmul(
                            py[:], lhsT=w_sb[:, kb * 128:(kb + 1) * 128],
                            rhs=rhs[:], start=(k == 0), stop=(k == 3))
                    eo = epool.tile([128, NWIN_GRP], F32, tag=f"eo{m}", name=f"eo{m}")
                    nc.scalar.activation(
                        eo[:], py[:], mybir.ActivationFunctionType.Silu,
                        bias=b_sb[:, m:m + 1])
                    nc.vector.tensor_add(eo[:], eo[:], nt32[m][:])
                    nc.sync.dma_start(yT[m, :, col0:col0 + NWIN_GRP], eo[:])

    nc.finalize()
    return nc


def _host_arrays(cfg, node_states, rel_states, W, b):
    N, H, OWN, NPAD = cfg["N"], cfg["H"], cfg["OWN"], cfg["NPAD"]
    node_states = np.asarray(node_states, dtype=np.float32)
    rel_states = np.asarray(rel_states, dtype=np.float32)
    W = np.asarray(W, dtype=np.float32)
    b = np.asarray(b, dtype=np.float32)

    tab_n = node_states.astype(ml_dtypes.bfloat16)
    tab_r = rel_states.astype(ml_dtypes.bfloat16)
    # W blocks: w_blk[p, (k*2+m)*128 + j] = W[k*128+p, m*128+j]
    w_blk = np.zeros((128, 8 * 128), dtype=ml_dtypes.bfloat16)
    for k in range(4):
        for m in range(2):
            kb = k * 2 + m
            w_blk[:, kb * 128:(kb + 1) * 128] = (
                W[k * 128:(k + 1) * 128, m * 128:(m + 1) * 128])
    b_blk = b.reshape(2, 128).T.astype(np.float32).copy()  # [128, 2]
    iota = np.tile(np.arange(128, dtype=np.float32)[None, :], (128, 1)
                   ).astype(ml_dtypes.bfloat16)

    in_maps = []
    for core in range(NCORES):
        lo = core * OWN
        hi = min(N, lo + OWN)
        slab = np.zeros((NPAD, H), dtype=np.float32)
        slab[: hi - lo] = node_states[lo:hi]
        sT = np.ascontiguousarray(slab.T)  # [H, NPAD]
        nd32 = sT.reshape(2, 128, NPAD)
        nd16 = nd32.astype(ml_dtypes.bfloat16)
        im = {
            "tab_n": tab_n, "tab_r": tab_r,
            "idx_n": cfg["streams"]["n"]["idx"][core],
            "idx_r": cfg["streams"]["r"]["idx"][core],
            "dcol_n": cfg["streams"]["n"]["dcol"][core],
            "dcol_r": cfg["streams"]["r"]["dcol"][core],
            "ndT16": nd16, "ndT32": nd32,
            "w_blk": w_blk, "b_blk": b_blk, "iota_d": iota,
        }
        in_maps.append(im)
    return in_maps


def kernel(node_states, rel_states, triples, W, b, _trace=False):
    cfg = _plan(node_states, rel_states, triples)
    nc = _build_program(cfg)
    in_maps = _host_arrays(cfg, node_states, rel_states, W, b)
    res = run_bass_kernel_spmd(nc, in_maps, core_ids=list(range(NCORES)),
                               trace=_trace)
    N, H, OWN, NPAD = cfg["N"], cfg["H"], cfg["OWN"], cfg["NPAD"]
    out = np.zeros((N, H), dtype=np.float32)
    for core in range(NCORES):
        yT = res.results[core]["yT"]  # [2, 128, NPAD]
        y = yT.reshape(H, NPAD).T    # [NPAD, H]
        lo = core * OWN
        hi = min(N, lo + OWN)
        out[lo:hi] = y[: hi - lo]
    if _trace:
        kernel.last_results = res
    return out



# revision 12
# speedup vs baseline: 1.1610x; 1.1610x over previous
"""Trainium2 Bass kernel for GraphTripletGCNLayer.

Reference computation (N=100000 nodes, R=100000 rels, T=300000 triples, H=256):
    rel = rel_states[rel_idx]
    agg = zeros; agg[obj] += node[subj] + rel; agg[subj] += node[obj] + rel
    out = node + silu(concat([node, agg]) @ W + b)

Strategy (8 cores, dst-node sharded). The binding resource is the GpSimd Q7
SWDGE descriptor generator (~7.5 ns per gathered row), so the kernel is
organized to (a) gather only real messages (runtime-count gathers skip all
padding), (b) issue few large gather calls, and (c) hide every other engine
behind the Pool-engine pipeline:
  - Each core owns a contiguous slab of 12500 destination nodes, processed
    in 25 groups of 512 dsts (4 windows x 128).
  - node/rel tables are replicated (bf16) in each core's DRAM; messages are
    host-sorted by (group, table-chunk) per stream. Each (stream, group,
    chunk) range is fetched with one gpsimd.dma_gather whose runtime count
    (register) covers only the real messages of this core.
  - Aggregation per group: for each 128-message tile, a [128, 512] onehot
    (DVE is_equal vs an fp16 iota) scatters the tile into PSUM via
    msg_tile^T @ onehot matmuls, accumulating agg^T feature-major.
  - Projection: y^T = W^T x^T over [node^T; agg^T] in bf16, silu(+bias) on
    ScalarE, bf16 residual add on DVE, bf16 feature-major output which the
    host transposes back and upcasts to f32.
"""

import sys, os

sys.path.insert(0, "/opt/trn_rl_repo")

import numpy as np
import ml_dtypes

import concourse.bass as bass
import concourse.bacc as bacc
import concourse.mybir as mybir
import concourse.tile as tile
from concourse.bass_utils import run_bass_kernel_spmd

BF16 = mybir.dt.bfloat16
FP16 = mybir.dt.float16
F32 = mybir.dt.float32
I16 = mybir.dt.int16
I32 = mybir.dt.int32

NCORES = 8
WIN = 128
GW = 4               # windows per group -> 512 dst columns in PSUM
GWIN = GW * WIN      # 512


def _ceil(a, b):
    return -(-a // b)


def _plan(node_states, rel_states, triples):
    """Host-side message planning. Returns cfg dict + per-core arrays."""
    N, H = node_states.shape
    R = rel_states.shape[0]
    assert H == 256, H

    OWN = _ceil(N, NCORES)            # 12500 owned dst nodes per core
    WPC = _ceil(OWN, WIN)             # 98 windows
    NG = _ceil(WPC, GW)               # 25 groups
    NPAD = NG * GWIN                  # 12800 padded node columns per core

    tr = np.asarray(triples).astype(np.int64)
    s, r, o = tr[:, 0], tr[:, 1], tr[:, 2]
    dst = np.concatenate([o, s])
    owner = dst // OWN
    dl = dst - owner * OWN
    g = dl // GWIN
    dstg = (dl - g * GWIN).astype(np.float32)   # dst-in-group, 0..511

    streams = {}
    for name, gidx, tabrows in (
        ("n", np.concatenate([s, o]), N),
        ("r", np.concatenate([r, r]), R),
    ):
        NCH = _ceil(tabrows, 32768)
        CH = _ceil(tabrows, NCH)
        chunk = gidx // CH
        lidx = (gidx - chunk * CH).astype(np.int16)
        cell = g * NCH + chunk                       # message cell id
        counts = np.zeros((NCORES, NG * NCH), dtype=np.int64)
        np.add.at(counts, (owner, cell), 1)
        P = 128 * np.maximum(_ceil(counts.max(axis=0), 128), 1)  # [NG*NCH]
        tile_base = np.concatenate([[0], np.cumsum(P // 128)[:-1]])
        slot_base = tile_base * 128
        T_tiles = int(P.sum()) // 128
        S = T_tiles * 128

        idx_cores = np.zeros((NCORES, 128, S // 16), dtype=np.int16)
        dcol_cores = np.full((NCORES, 128, T_tiles), -1.0, dtype=np.float32)
        cnt_cores = np.zeros((NCORES, NG * NCH), dtype=np.int32)
        for core in range(NCORES):
            m = owner == core
            mc = cell[m]
            ml = lidx[m]
            md = dstg[m]
            srt = np.argsort(mc, kind="stable")
            mc = mc[srt]
            ml = ml[srt]
            md = md[srt]
            starts = np.searchsorted(mc, np.arange(NG * NCH))
            pos = np.arange(mc.size) - starts[mc]
            slots = slot_base[mc] + pos
            idx_flat = np.full(S, -1, dtype=np.int16)
            d_flat = np.full(S, -1.0, dtype=np.float32)
            idx_flat[slots] = ml
            d_flat[slots] = md
            # Runtime gather counts: multiple of 16, >= 32, and the ucode
            # contract requires count == #non-negative idxs in the window,
            # so slots [real, cnt) get a valid dummy index (row 0).
            cnt = np.maximum(counts[core], 32)
            cnt = ((cnt + 15) // 16) * 16
            cnt = np.minimum(cnt, P)
            # groups 0-1 are gathered with static full counts to overwrite
            # virgin SBUF: every slot must hold a valid index there.
            g_of_cell = np.arange(NG * NCH) // NCH
            cnt = np.where(g_of_cell < 2, P, cnt)
            for cell_id in range(NG * NCH):
                lo = slot_base[cell_id] + counts[core, cell_id]
                hi = slot_base[cell_id] + cnt[cell_id]
                if hi > lo:
                    idx_flat[lo:hi] = 0
            cnt_cores[core] = cnt.astype(np.int32)
            idx_cores[core] = np.tile(idx_flat.reshape(-1, 16).T, (8, 1))
            dcol_cores[core] = d_flat.reshape(T_tiles, 128).T

        streams[name] = dict(
            NCH=NCH, CH=CH, P=P, tile_base=tile_base, T_tiles=T_tiles, S=S,
            idx=idx_cores, dcol=dcol_cores, cnt=cnt_cores,
        )

    cfg = dict(N=N, R=R, H=H, OWN=OWN, WPC=WPC, NG=NG, NPAD=NPAD,
               streams=streams)
    return cfg


def _build_program(cfg):
    N, R, H = cfg["N"], cfg["R"], cfg["H"]
    NG, NPAD = cfg["NG"], cfg["NPAD"]
    stn, str_ = cfg["streams"]["n"], cfg["streams"]["r"]

    nc = bacc.Bacc("TRN2", target_bir_lowering=False, debug=False)

    tab_n = nc.dram_tensor("tab_n", [N, H], BF16, kind="ExternalInput")
    tab_r = nc.dram_tensor("tab_r", [R, H], BF16, kind="ExternalInput")
    idx_n = nc.dram_tensor("idx_n", [128, stn["S"] // 16], I16,
                           kind="ExternalInput")
    idx_r = nc.dram_tensor("idx_r", [128, str_["S"] // 16], I16,
                           kind="ExternalInput")
    dcol_n = nc.dram_tensor("dcol_n", [128, stn["T_tiles"]], F32,
                            kind="ExternalInput")
    dcol_r = nc.dram_tensor("dcol_r", [128, str_["T_tiles"]], F32,
                            kind="ExternalInput")
    ncalls = stn["P"].size + str_["P"].size
    CNT_PAD = _ceil(ncalls, 16) * 16
    cnts_t = nc.dram_tensor("cnts", [1, CNT_PAD], I32, kind="ExternalInput")
    ndT = nc.dram_tensor("ndT", [128, 2, NPAD], BF16, kind="ExternalInput")
    w_blk = nc.dram_tensor("w_blk", [128, 8 * 128], BF16, kind="ExternalInput")
    b_blk = nc.dram_tensor("b_blk", [128, 2], F32, kind="ExternalInput")
    iota_d = nc.dram_tensor("iota_d", [128, GWIN], FP16, kind="ExternalInput")
    yT = nc.dram_tensor("yT", [128, 2, NPAD], BF16, kind="ExternalOutput")

    dram = dict(n=(tab_n, idx_n, dcol_n, N), r=(tab_r, idx_r, dcol_r, R))
    # tiles per group per stream (static shapes shared across groups)
    tpg = {}
    for sname, st in (("n", stn), ("r", str_)):
        NCH = st["NCH"]
        per_g = [int(st["P"][g * NCH:(g + 1) * NCH].sum()) // 128
                 for g in range(NG)]
        tpg[sname] = max(per_g)

    with tile.TileContext(nc) as tc:
        with (
            tc.tile_pool(name="const", bufs=1) as cpool,
            tc.tile_pool(name="meta", bufs=1) as mpool,
            tc.tile_pool(name="gath", bufs=2) as gpool,
            tc.tile_pool(name="oh", bufs=12) as ohpool,
            tc.tile_pool(name="asb", bufs=2) as apool,
            tc.tile_pool(name="ndt", bufs=2) as npool,
            tc.tile_pool(name="eout", bufs=2) as epool,
            tc.tile_pool(name="pswin", bufs=2, space="PSUM") as pswin,
            tc.tile_pool(name="psy", bufs=2, space="PSUM") as psy,
        ):
            iota_sb = cpool.tile([128, GWIN], FP16)
            nc.sync.dma_start(iota_sb[:], iota_d[:])
            w_sb = cpool.tile([128, 8 * 128], BF16)
            nc.sync.dma_start(w_sb[:], w_blk[:])
            b_sb = cpool.tile([128, 2], F32)
            nc.sync.dma_start(b_sb[:], b_blk[:])
            cnt_sb = mpool.tile([1, CNT_PAD], I32, tag="cnt", name="cnt")
            nc.sync.dma_start(cnt_sb[:], cnts_t[:])

            meta = {}
            for sname, st in (("n", stn), ("r", str_)):
                _, idx_t, dcol_t, _ = dram[sname]
                idx_sb = mpool.tile([128, st["S"] // 16], I16,
                                    tag=f"idx{sname}", name=f"idx{sname}")
                nc.sync.dma_start(idx_sb[:], idx_t[:])
                dcol_sb = mpool.tile([128, st["T_tiles"]], F32,
                                     tag=f"dc{sname}", name=f"dc{sname}")
                nc.sync.dma_start(dcol_sb[:], dcol_t[:])
                meta[sname] = (idx_sb, dcol_sb)

            cregs = [nc.gpsimd.alloc_register(name=f"creg{i}") for i in range(16)]
            regi = [0]

            for g in range(NG):
                # ---- gathers for this group (runtime counts) ----
                gt = {}
                for sname, st in (("n", stn), ("r", str_)):
                    tab_t, _, _, tabrows = dram[sname]
                    idx_sb, _ = meta[sname]
                    NCH, CH, P = st["NCH"], st["CH"], st["P"]
                    gtile = gpool.tile([128, tpg[sname], H], BF16,
                                       tag=f"g{sname}", name=f"g{sname}")
                    lt = 0
                    for c in range(NCH):
                        cell = g * NCH + c
                        Pc = int(P[cell])
                        s0 = int(st["tile_base"][cell]) * 128
                        rows0 = c * CH
                        rows1 = min(CH * (c + 1), tabrows)
                        if (g < 2 or g >= 2 + int(os.environ.get("K_RT", "99"))
                                or os.environ.get("K_STATIC", "0") == "1"):
                            cnt_arg = Pc   # virgin SBUF: gather padding too
                        else:
                            k = cell if sname == "n" else stn["P"].size + cell
                            creg = cregs[regi[0] % 16]
                            regi[0] += 1
                            nc.gpsimd.reg_load(creg, cnt_sb[0:1, k:k + 1])
                            cnt_arg = creg
                        nc.gpsimd.dma_gather(
                            gtile[:, lt:lt + Pc // 128, :],
                            tab_t[rows0:rows1, :],
                            idx_sb[:, s0 // 16:(s0 + Pc) // 16],
                            Pc, cnt_arg, H,
                            single_packet=os.environ.get("K_SP", "0") == "1")
                        lt += Pc // 128
                    gt[sname] = gtile

                # ---- scatter matmuls into group PSUM ----
                pw = [pswin.tile([128, GWIN], F32, tag=f"pw{m}",
                                 name=f"pw{m}") for m in range(2)]
                tl = []
                for sname, st in (("n", stn), ("r", str_)):
                    NCH, P = st["NCH"], st["P"]
                    lt = 0
                    for c in range(NCH):
                        cell = g * NCH + c
                        tb = int(st["tile_base"][cell])
                        for t in range(int(P[cell]) // 128):
                            tl.append((sname, lt + t, tb + t))
                        lt += int(P[cell]) // 128
                for i, (sname, lt, gidx) in enumerate(tl):
                    _, dcol_sb = meta[sname]
                    oh = ohpool.tile([128, GWIN], BF16, tag="oh")
                    nc.vector.tensor_scalar(
                        oh[:], iota_sb[:], dcol_sb[:, gidx:gidx + 1],
                        None, mybir.AluOpType.is_equal)
                    for m in range(2):
                        nc.tensor.matmul(
                            pw[m][:],
                            lhsT=gt[sname][:, lt, m * 128:(m + 1) * 128],
                            rhs=oh[:],
                            start=(i == 0), stop=(i == len(tl) - 1))

                # ---- epilogue: copy agg, project, silu, residual ----
                at = apool.tile([128, 2, GWIN], BF16, tag="at", name="at")
                for m in range(2):
                    nc.scalar.activation(
                        at[:, m, :], pw[m][:],
                        mybir.ActivationFunctionType.Copy)
                col0 = g * GWIN
                nt = npool.tile([128, 2, GWIN], BF16, tag="nt", name="nt")
                nc.sync.dma_start(nt[:], ndT[:, :, col0:col0 + GWIN])
                eo = epool.tile([128, 2, GWIN], BF16, tag="eo", name="eo")
                for m in range(2):
                    py = psy.tile([128, GWIN], F32)
                    for k in range(4):
                        rhs = nt[:, k, :] if k < 2 else at[:, k - 2, :]
                        kb = k * 2 + m
                        nc.tensor.matmul(
                            py[:], lhsT=w_sb[:, kb * 128:(kb + 1) * 128],
                            rhs=rhs, start=(k == 0), stop=(k == 3))
                    nc.scalar.activation(
                        eo[:, m, :], py[:], mybir.ActivationFunctionType.Silu,
                        bias=b_sb[:, m:m + 1])
                nc.vector.tensor_tensor(
                    eo[:], eo[:], nt[:], mybir.AluOpType.add)
                nc.sync.dma_start(yT[:, :, col0:col0 + GWIN], eo[:])

    nc.finalize()
    return nc


def _host_arrays(cfg, node_states, rel_states, W, b):
    N, H, OWN, NPAD = cfg["N"], cfg["H"], cfg["OWN"], cfg["NPAD"]
    node_states = np.asarray(node_states, dtype=np.float32)
    rel_states = np.asarray(rel_states, dtype=np.float32)
    W = np.asarray(W, dtype=np.float32)
    b = np.asarray(b, dtype=np.float32)

    tab_n = node_states.astype(ml_dtypes.bfloat16)
    tab_r = rel_states.astype(ml_dtypes.bfloat16)
    # W blocks: w_blk[p, (k*2+m)*128 + j] = W[k*128+p, m*128+j]
    w_blk = np.zeros((128, 8 * 128), dtype=ml_dtypes.bfloat16)
    for k in range(4):
        for m in range(2):
            kb = k * 2 + m
            w_blk[:, kb * 128:(kb + 1) * 128] = (
                W[k * 128:(k + 1) * 128, m * 128:(m + 1) * 128])
    b_blk = b.reshape(2, 128).T.astype(np.float32).copy()  # [128, 2]
    iota = np.tile(np.arange(GWIN, dtype=np.float32)[None, :], (128, 1)
                   ).astype(np.float16)

    stn, str_ = cfg["streams"]["n"], cfg["streams"]["r"]
    ncalls = stn["P"].size + str_["P"].size
    CNT_PAD = _ceil(ncalls, 16) * 16

    in_maps = []
    for core in range(NCORES):
        lo = core * OWN
        hi = min(N, lo + OWN)
        slab = np.zeros((NPAD, H), dtype=np.float32)
        slab[: hi - lo] = node_states[lo:hi]
        ndT = np.ascontiguousarray(
            slab.T.reshape(2, 128, NPAD).transpose(1, 0, 2)
        ).astype(ml_dtypes.bfloat16)
        cnts = np.zeros((1, CNT_PAD), dtype=np.int32)
        cnts[0, :stn["P"].size] = stn["cnt"][core]
        cnts[0, stn["P"].size:ncalls] = str_["cnt"][core]
        im = {
            "tab_n": tab_n, "tab_r": tab_r,
            "idx_n": stn["idx"][core],
            "idx_r": str_["idx"][core],
            "dcol_n": stn["dcol"][core],
            "dcol_r": str_["dcol"][core],
            "cnts": cnts,
            "ndT": ndT,
            "w_blk": w_blk, "b_blk": b_blk, "iota_d": iota,
        }
        in_maps.append(im)
    return in_maps


def kernel(node_states, rel_states, triples, W, b, _trace=False):
    cfg = _plan(node_states, rel_states, triples)
    nc = _build_program(cfg)
    in_maps = _host_arrays(cfg, node_states, rel_states, W, b)
    res = run_bass_kernel_spmd(nc, in_maps, core_ids=list(range(NCORES)),
                               trace=_trace)
    N, H, OWN, NPAD = cfg["N"], cfg["H"], cfg["OWN"], cfg["NPAD"]
    out = np.zeros((N, H), dtype=np.float32)
    for core in range(NCORES):
        yT = res.results[core]["yT"]              # [128, 2, NPAD] bf16
        y = yT.transpose(1, 0, 2).reshape(H, NPAD).T.astype(np.float32)
        lo = core * OWN
        hi = min(N, lo + OWN)
        out[lo:hi] = y[: hi - lo]
    if _trace:
        kernel.last_results = res
    return out
